# revision 1
# baseline (speedup 1.0000x reference)
"""GraphSAGE (gnn_message_passing) forward pass on 8 Trainium2 NeuronCores.

Sharding strategy (hardcoded): row-shard the 10000 nodes across 8 cores
(1250 each, padded to 1280).  Each core receives its shard of the adjacency
matrix pre-transposed ([10240, 1280] bf16, columns = this core's nodes) so
the aggregation matmuls stream contiguous natural-layout slabs.  Node
features are kept feature-major ([feat_part, node_free]) on-chip so every
linear layer is a natural matmul.  Between GNN layers the updated node
features are AllGathered (bf16, node-major) so every core sees all nodes
for the next aggregation.  Small weights / LSTM params are replicated.
"""

import os
from contextlib import ExitStack

import numpy as np
import ml_dtypes

import concourse.bass as bass
import concourse.bacc as bacc
import concourse.mybir as mybir
import concourse.tile as tile
from concourse.bass_utils import run_bass_kernel_spmd
from concourse.masks import make_identity

F32 = mybir.dt.float32
BF16 = mybir.dt.bfloat16
AX = mybir.AxisListType
OP = mybir.AluOpType
AF = mybir.ActivationFunctionType

# ---- problem constants (hardcoded per spec) ----
N = 10000        # nodes
NC = 8           # cores
NPC = 1250       # original nodes per core
PC = 1280        # padded nodes per core
NP = NC * PC     # padded total nodes = 10240
KT = NP // 128   # 80 contraction tiles
IT = PC // 128   # 10 node tiles per core
NFEAT = 2000
FPAD = 2048
FT = FPAD // 128  # 16
NH = 128
NHE = 64
NFE = 256
D = NH + NHE     # 192
NOUT = 20
L = 2
BN_EPS = 1e-5

# matmul free-dim chunks over PC (PSUM bank = 512 fp32)
CHUNKS = [(0, 512), (512, 512), (1024, 256)]

LAST_RESULT = None  # test.py reads exec_time info from here

_CACHED_NC = None


def _bf(a):
    return np.asarray(a, dtype=ml_dtypes.bfloat16)


def _f32(a):
    return np.ascontiguousarray(a, dtype=np.float32)


# --------------------------------------------------------------------------
# device program
# --------------------------------------------------------------------------

def _build_program():
    nc = bacc.Bacc("TRN2", target_bir_lowering=False, debug=False, num_devices=NC)

    def inp(name, shape, dtype):
        return nc.declare_dram_parameter(name, list(shape), dtype, isOutput=False)

    # per-core tensors
    d_adjT = inp("adjT", [NP, PC], BF16)
    d_xT = inp("xT", [FPAD, PC], BF16)
    d_embT = inp("embT", [NFE, PC], F32)
    d_rsb = inp("rsb", [128, PC], F32)          # 1/rowsum broadcast to 128 parts
    # replicated weights
    d_w_inT = inp("w_inT", [FPAD, NH], BF16)
    d_sc_in = inp("sc_in", [NH, 1], F32)
    d_sh_in = inp("sh_in", [NH, 1], F32)        # with b_in folded
    d_sc_in_h = inp("sc_in_h", [NH, 1], F32)    # 0.5 * sc_in (for JK mean)
    d_sh_in2 = inp("sh_in2", [NH, 1], F32)      # without linear bias
    d_wgs_sT = inp("wgs_sT", [L, NH, NH], F32)
    d_wgs_nT = inp("wgs_nT", [L, NH, NH], F32)
    d_bgs = inp("bgs", [NH, L], F32)
    d_wihT = inp("wihT", [L, NH, 4 * NH], BF16)
    d_whhT = inp("whhT", [L, NH, 4 * NH], BF16)
    d_blstm = inp("blstm", [NH, 2 * 4], F32)    # (l, gate) on free dim
    d_w_embT = inp("w_embT", [NFE, NHE], F32)
    d_sc_emb = inp("sc_emb", [NHE, 1], F32)
    d_sh_emb = inp("sh_emb", [NHE, 1], F32)
    d_w_fcT = inp("w_fcT", [D, D], F32)
    d_sc_fc_a = inp("sc_fc_a", [128, 1], F32)
    d_sh_fc_a = inp("sh_fc_a", [128, 1], F32)
    d_sc_fc_b = inp("sc_fc_b", [64, 1], F32)
    d_sh_fc_b = inp("sh_fc_b", [64, 1], F32)
    d_w_outT = inp("w_outT", [D, NOUT], F32)
    d_bout = inp("bout", [128, NOUT], F32)      # b_out broadcast across parts
    d_out = nc.declare_dram_parameter("out", [PC, NOUT], F32, isOutput=True)

    # internal DRAM for collectives (each gather split in 2 halves so the
    # second half's transfer overlaps aggregation on the first half)
    HT = IT // 2   # 5 local k-tiles per half
    bounce = [[nc.dram_tensor(f"bounce{l}_{h}", [HT, 128, 128], BF16)
               for h in range(2)] for l in range(L)]
    hg = [[nc.dram_tensor(f"hg{l}_{h}", [NC * HT, 128, 128], BF16,
                          addr_space="Shared") for h in range(2)]
          for l in range(L)]
    groups = [list(range(NC))]

    with tile.TileContext(nc) as tc, ExitStack() as top:
        const = top.enter_context(tc.tile_pool(name="const", bufs=1))
        persist = top.enter_context(tc.tile_pool(name="persist", bufs=1))
        tmpf = top.enter_context(tc.tile_pool(name="tmpf", bufs=1))
        slabp = top.enter_context(tc.tile_pool(name="slab", bufs=4))
        hnatp = top.enter_context(tc.tile_pool(name="hnat", bufs=1))

        # ---- load constants ----
        w_in = const.tile([128, FPAD], BF16, tag="w_in")
        nc.sync.dma_start(w_in.rearrange("p (t j) -> p t j", t=FT),
                          d_w_inT.ap().rearrange("(t p) j -> p t j", p=128))
        wgs_s = [const.tile([128, NH], F32, tag=f"wgs_s{l}", name=f"wgs_s{l}")
                 for l in range(L)]
        wgs_n = [const.tile([128, NH], F32, tag=f"wgs_n{l}", name=f"wgs_n{l}")
                 for l in range(L)]
        for l in range(L):
            nc.sync.dma_start(wgs_s[l], d_wgs_sT[l])
            nc.sync.dma_start(wgs_n[l], d_wgs_nT[l])
        bgs = const.tile([128, L], F32, tag="bgs")
        nc.sync.dma_start(bgs, d_bgs.ap())
        wih = [const.tile([128, 4 * NH], BF16, tag=f"wih{l}", name=f"wih{l}")
               for l in range(L)]
        whh = [const.tile([128, 4 * NH], BF16, tag=f"whh{l}", name=f"whh{l}")
               for l in range(L)]
        for l in range(L):
            nc.sync.dma_start(wih[l], d_wihT[l])
            nc.sync.dma_start(whh[l], d_whhT[l])
        blstm = const.tile([128, 8], F32, tag="blstm")
        nc.sync.dma_start(blstm, d_blstm.ap())
        w_emb = [const.tile([128, NHE], F32, tag=f"w_emb{t}", name=f"w_emb{t}")
                 for t in range(2)]
        for t in range(2):
            nc.sync.dma_start(w_emb[t], d_w_embT[t * 128:(t + 1) * 128, :])
        wfc_aa = const.tile([128, 128], F32, tag="wfc_aa")
        wfc_ba = const.tile([64, 128], F32, tag="wfc_ba")
        wfc_ab = const.tile([128, 64], F32, tag="wfc_ab")
        wfc_bb = const.tile([64, 64], F32, tag="wfc_bb")
        nc.sync.dma_start(wfc_aa, d_w_fcT[:128, :128])
        nc.sync.dma_start(wfc_ba, d_w_fcT[128:, :128])
        nc.sync.dma_start(wfc_ab, d_w_fcT[:128, 128:])
        nc.sync.dma_start(wfc_bb, d_w_fcT[128:, 128:])
        w_out_a = const.tile([128, NOUT], F32, tag="w_out_a")
        w_out_b = const.tile([64, NOUT], F32, tag="w_out_b")
        nc.sync.dma_start(w_out_a, d_w_outT[:128, :])
        nc.sync.dma_start(w_out_b, d_w_outT[128:, :])
        bout = const.tile([128, NOUT], F32, tag="bout")
        nc.sync.dma_start(bout, d_bout.ap())
        rsb = const.tile([128, PC], F32, tag="rsb")
        nc.sync.dma_start(rsb, d_rsb.ap())

        small = {}
        for nm, dd, p in [
            ("sc_in", d_sc_in, NH), ("sh_in", d_sh_in, NH),
            ("sc_in_h", d_sc_in_h, NH), ("sh_in2", d_sh_in2, NH),
            ("sc_emb", d_sc_emb, NHE), ("sh_emb", d_sh_emb, NHE),
            ("sc_fc_a", d_sc_fc_a, 128), ("sh_fc_a", d_sh_fc_a, 128),
            ("sc_fc_b", d_sc_fc_b, 64), ("sh_fc_b", d_sh_fc_b, 64),
        ]:
            t = const.tile([p, 1], F32, tag=nm, name=nm)
            nc.sync.dma_start(t, dd.ap())
            small[nm] = t

        ident_bf = const.tile([128, 128], BF16, tag="ident")
        make_identity(nc, ident_bf)
        ones_col = const.tile([128, 1], F32, tag="ones_col")
        nc.vector.memset(ones_col, 1.0)
        ones_row = const.tile([1, 128], F32, tag="ones_row")
        nc.vector.memset(ones_row, 1.0)

        # ---- helpers ----
        def mm_acc(psum_ap, lhsT, rhs, start, stop):
            """accumulate lhsT.T @ rhs into psum, chunking free dim at 512"""
            F = rhs.shape[-1]
            o = 0
            while o < F:
                w = min(512, F - o)
                nc.tensor.matmul(
                    psum_ap[:, o:o + w], lhsT, rhs[:, o:o + w],
                    start=start, stop=stop,
                )
                o += w

        def elu_from(out_sb, in_ap, sc_ap, sh_ap):
            """out = elu(sc*in + sh); in_ap may be PSUM; [P, F]"""
            P, F = out_sb.shape[0], out_sb.shape[-1]
            y = tmpf.tile([128, PC], F32, tag="elu_y", name="elu_y")[:P, :F]
            nc.vector.tensor_scalar(y, in_ap, sc_ap, sh_ap, OP.mult, OP.add)
            e = tmpf.tile([128, PC], F32, tag="elu_e", name="elu_e")[:P, :F]
            nc.vector.tensor_scalar_min(e, y, 0.0)
            nc.scalar.activation(e, e, AF.Exp)
            # y <- max(y,0) - 1   (in place)
            nc.vector.tensor_scalar(y, y, 0.0, -1.0, OP.max, OP.add)
            nc.vector.tensor_tensor(out_sb, y, e, OP.add)

        # persistent activations
        hT = [persist.tile([128, PC], F32, tag="hT", bufs=2, name=f"hT{l}")
              for l in range(3)]
        hT_bf = [persist.tile([128, PC], BF16, tag=f"hTbf{l}", name=f"hTbf{l}")
                 for l in range(3)]

        with tc.tile_pool(name="psA", bufs=1, space="PSUM") as psA, \
             tc.tile_pool(name="psS", bufs=1, space="PSUM") as psS, \
             tc.tile_pool(name="psB", bufs=1, space="PSUM") as psB, \
             tc.tile_pool(name="psT", bufs=2, space="PSUM") as psT, \
             tc.tile_pool(name="tmpc", bufs=2) as tmpc:

            def gather(l, src_bf):
                """transpose local hT bf16 to node-major, AllGather (2 halves)"""
                for h in range(2):
                    loc = tmpc.tile([128, HT * 128], BF16, tag="hnat_loc",
                                    name="hnat_loc")
                    for s in range(HT):
                        it = h * HT + s
                        pt = psT.tile([128, 128], BF16, tag="tp", name="tp")
                        nc.tensor.transpose(
                            pt, src_bf[:, it * 128:(it + 1) * 128], ident_bf)
                        nc.vector.tensor_copy(loc[:, s * 128:(s + 1) * 128], pt)
                    nc.sync.dma_start(
                        bounce[l][h].ap().rearrange("t p f -> p t f"),
                        loc.rearrange("p (t f) -> p t f", t=HT))
                    nc.gpsimd.collective_compute(
                        "AllGather", OP.bypass, replica_groups=groups,
                        ins=[bounce[l][h].ap().opt()],
                        outs=[hg[l][h].ap().opt()],
                    )

            # ---- input projection: h0T = elu(bn(W_in @ x)) ----
            ps = psA.tile([128, PC], F32, tag="big", name="ps_proj")
            for t in range(FT):
                xt = tmpf.tile([128, PC], BF16, tag="xstream", bufs=3,
                               name="xt")
                nc.sync.dma_start(xt, d_xT[t * 128:(t + 1) * 128, :])
                mm_acc(ps, w_in[:, t * 128:(t + 1) * 128], xt,
                       start=(t == 0), stop=(t == FT - 1))
            elu_from(hT[0], ps, small["sc_in"], small["sh_in"])
            nc.vector.tensor_copy(hT_bf[0], hT[0])
            gather(0, hT_bf[0])

            # ---- GNN layers ----
            for l in range(L):
                ps_agg = psA.tile([128, PC], F32, tag="big", name="ps_agg")
                for h in range(2):
                    hnat = hnatp.tile([128, NC * HT * 128], BF16, tag="hnat",
                                      bufs=2, name="hnat")
                    nc.sync.dma_start(
                        hnat.rearrange("p (t f) -> p t f", t=NC * HT),
                        hg[l][h].ap().rearrange("t p f -> p t f"))
                    for r in range(NC):
                        row0 = r * PC + h * HT * 128
                        slab = slabp.tile([128, HT, PC], BF16, tag="slab",
                                          bufs=2, name="slab")
                        nc.sync.dma_start(
                            slab,
                            d_adjT[row0:row0 + HT * 128, :].rearrange(
                                "(s p) i -> p s i", p=128))
                        for s in range(HT):
                            t = r * HT + s
                            mm_acc(ps_agg, hnat[:, t * 128:(t + 1) * 128],
                                   slab[:, s, :],
                                   start=(h == 0 and r == 0 and s == 0),
                                   stop=(h == 1 and r == NC - 1 and s == HT - 1))
                neighT = tmpf.tile([128, PC], F32, tag="neighT", name="neighT")
                nc.vector.tensor_tensor(neighT, ps_agg, rsb, OP.mult)

                # GS linear: relu(W_self @ h + W_neigh @ neigh + b)
                ps_gs = psA.tile([128, PC], F32, tag="big", name="ps_gs")
                mm_acc(ps_gs, wgs_s[l], hT[l], start=True, stop=False)
                mm_acc(ps_gs, wgs_n[l], neighT, start=False, stop=True)
                hrelu = tmpf.tile([128, PC], F32, tag="hrelu", name="hrelu")
                nc.scalar.activation(hrelu, ps_gs, AF.Relu,
                                     bias=bgs[:, l:l + 1], scale=1.0)

                # L2 normalize along features (partition dim) via PE ones-reduce
                sq = tmpf.tile([128, PC], F32, tag="sq", name="sq")
                nc.vector.tensor_tensor(sq, hrelu, hrelu, OP.mult)
                nrm = tmpf.tile([1, PC], F32, tag="nrm", name="nrm")
                for (o, w) in CHUNKS:
                    ps_ss = psS.tile([1, 512], F32, tag="ss", name="ps_ss")
                    nc.tensor.matmul(ps_ss[:, :w], ones_col, sq[:, o:o + w],
                                     start=True, stop=True)
                    nc.scalar.activation(nrm[:, o:o + w], ps_ss[:, :w], AF.Sqrt)
                nc.vector.tensor_scalar_max(nrm, nrm, 1e-12)
                rec = tmpf.tile([1, PC], F32, tag="rec", name="rec")
                nc.vector.reciprocal(rec, nrm)
                for (o, w) in CHUNKS:
                    ps_bc = psB.tile([128, 512], F32, tag="bc", name="ps_bc")
                    nc.tensor.matmul(ps_bc[:, :w], ones_row, rec[:, o:o + w],
                                     start=True, stop=True)
                    nc.vector.tensor_tensor(hT[l + 1][:, o:o + w],
                                            hrelu[:, o:o + w], ps_bc[:, :w],
                                            OP.mult)
                nc.vector.tensor_copy(hT_bf[l + 1], hT[l + 1])
                if l == 0:
                    gather(1, hT_bf[1])

        # ---- 2-layer LSTM jumping knowledge over T=2 ----
        c_st = [persist.tile([128, PC], F32, tag=f"c{l}", name=f"c{l}")
                for l in range(2)]
        o_bf = [persist.tile([128, PC], BF16, tag=f"o{t}", name=f"o{t}")
                for t in range(2)]
        p0_bf = persist.tile([128, PC], BF16, tag="p0bf")
        p_f = [persist.tile([128, PC], F32, tag=f"p{t}f", name=f"p{t}f")
               for t in range(2)]

        with tc.tile_pool(name="psL", bufs=2, space="PSUM") as psL, \
             tc.tile_pool(name="tmpg", bufs=1) as tmpg:

            def lstm_cell(l, t, xin_bf, hprev_bf, c_tile, out_f32, out_bf):
                for (o, w) in CHUNKS:
                    gps = [psL.tile([128, 512], F32, tag=f"g{g}", name=f"g{g}")
                           for g in range(4)]
                    for g in range(4):
                        nc.tensor.matmul(
                            gps[g][:, :w],
                            wih[l][:, g * 128:(g + 1) * 128],
                            xin_bf[:, o:o + w],
                            start=True, stop=(t == 0))
                        if t > 0:
                            nc.tensor.matmul(
                                gps[g][:, :w],
                                whh[l][:, g * 128:(g + 1) * 128],
                                hprev_bf[:, o:o + w],
                                start=False, stop=True)
                    gact = []
                    for g, fn in enumerate([AF.Sigmoid, AF.Sigmoid,
                                            AF.Tanh, AF.Sigmoid]):
                        gt = tmpg.tile([128, 512], F32, tag=f"ga{g}",
                                       name=f"ga{g}")[:, :w]
                        nc.scalar.activation(gt, gps[g][:, :w], fn,
                                             bias=blstm[:, l * 4 + g:l * 4 + g + 1])
                        gact.append(gt)
                    ig, fg, gg, og = gact
                    cs = c_tile[:, o:o + w]
                    if t == 0:
                        nc.vector.tensor_tensor(cs, ig, gg, OP.mult)
                    else:
                        fc_ = tmpg.tile([128, 512], F32, tag="fc_",
                                        name="fc_")[:, :w]
                        nc.vector.tensor_tensor(fc_, fg, cs, OP.mult)
                        igg = tmpg.tile([128, 512], F32, tag="igg",
                                        name="igg")[:, :w]
                        nc.vector.tensor_tensor(igg, ig, gg, OP.mult)
                        nc.vector.tensor_tensor(cs, fc_, igg, OP.add)
                    tc_ = tmpg.tile([128, 512], F32, tag="tc_",
                                    name="tc_")[:, :w]
                    nc.scalar.activation(tc_, cs, AF.Tanh)
                    if out_f32 is not None:
                        nc.vector.tensor_tensor(out_f32[:, o:o + w], og, tc_,
                                                OP.mult)
                        if out_bf is not None:
                            nc.vector.tensor_copy(out_bf[:, o:o + w],
                                                  out_f32[:, o:o + w])
                    else:
                        nc.vector.tensor_tensor(out_bf[:, o:o + w], og, tc_,
                                                OP.mult)

            # layer0 t0; layer1 t0; layer0 t1; layer1 t1
            lstm_cell(0, 0, hT_bf[1], None, c_st[0], None, o_bf[0])
            lstm_cell(1, 0, o_bf[0], None, c_st[1], p_f[0], p0_bf)
            lstm_cell(0, 1, hT_bf[2], o_bf[0], c_st[0], None, o_bf[1])
            lstm_cell(1, 1, o_bf[1], p0_bf, c_st[1], p_f[1], None)

        # ---- post: JK mean -> bn/elu ; embed ; fc ; logits ; log_softmax ----
        hpost = persist.tile([128, PC], F32, tag="hpost")
        eT = persist.tile([64, PC], F32, tag="eT")
        hfca = persist.tile([128, PC], F32, tag="hfca")
        hfcb = persist.tile([64, PC], F32, tag="hfcb")
        outall = persist.tile([128, IT * NOUT], F32, tag="outall")

        with tc.tile_pool(name="psP", bufs=2, space="PSUM") as psP, \
             tc.tile_pool(name="psG", bufs=2, space="PSUM") as psG, \
             tc.tile_pool(name="tmps", bufs=2) as tmps:

            hsum = tmpf.tile([128, PC], F32, tag="neighT", name="hsum")
            nc.vector.tensor_tensor(hsum, p_f[0], p_f[1], OP.add)
            # 0.5 from the mean is folded into sc_in_h
            elu_from(hpost, hsum, small["sc_in_h"], small["sh_in2"])

            # embed projection
            ps_e = psP.tile([128, PC], F32, tag="post", name="ps_e")
            for t in range(2):
                et = tmpf.tile([128, PC], F32, tag="sq", name="et")
                nc.sync.dma_start(et, d_embT[t * 128:(t + 1) * 128, :])
                mm_acc(ps_e[:64, :], w_emb[t], et, start=(t == 0), stop=(t == 1))
            elu_from(eT, ps_e[:64, :], small["sc_emb"], small["sh_emb"])

            # fc on concat([hpost, eT]) without materializing the concat
            ps_fa = psP.tile([128, PC], F32, tag="post", name="ps_fa")
            mm_acc(ps_fa, wfc_aa, hpost, start=True, stop=False)
            mm_acc(ps_fa, wfc_ba, eT, start=False, stop=True)
            elu_from(hfca, ps_fa, small["sc_fc_a"], small["sh_fc_a"])
            ps_fb = psP.tile([128, PC], F32, tag="post", name="ps_fb")
            mm_acc(ps_fb[:64, :], wfc_ab, hpost, start=True, stop=False)
            mm_acc(ps_fb[:64, :], wfc_bb, eT, start=False, stop=True)
            elu_from(hfcb, ps_fb[:64, :], small["sc_fc_b"], small["sh_fc_b"])

            # logits per node-tile (natural orientation) + log_softmax
            for it in range(IT):
                ps_lg = psG.tile([128, NOUT], F32, tag="lg", name="ps_lg")
                nc.tensor.matmul(ps_lg, hfca[:, it * 128:(it + 1) * 128],
                                 w_out_a, start=True, stop=False)
                nc.tensor.matmul(ps_lg, hfcb[:, it * 128:(it + 1) * 128],
                                 w_out_b, start=False, stop=True)
                lg = tmps.tile([128, NOUT], F32, tag="lg_sb", name="lg_sb")
                nc.vector.tensor_tensor(lg, ps_lg, bout, OP.add)
                mx = tmps.tile([128, 1], F32, tag="mx", name="mx")
                nc.vector.tensor_reduce(mx, lg, AX.X, OP.max)
                sh = tmps.tile([128, NOUT], F32, tag="shift", name="shifted")
                nc.vector.tensor_scalar(sh, lg, mx, None, OP.subtract)
                ex = tmps.tile([128, NOUT], F32, tag="ex", name="ex")
                se = tmps.tile([128, 1], F32, tag="se", name="se")
                nc.scalar.activation(ex, sh, AF.Exp, accum_out=se)
                lse = tmps.tile([128, 1], F32, tag="lse", name="lse")
                nc.scalar.activation(lse, se, AF.Ln)
                nc.vector.tensor_scalar(
                    outall[:, it * NOUT:(it + 1) * NOUT], sh, lse, None,
                    OP.subtract)

            nc.sync.dma_start(
                d_out.ap().rearrange("(t p) c -> p t c", p=128),
                outall.rearrange("p (t c) -> p t c", t=IT))

    nc.compile()
    return nc


# --------------------------------------------------------------------------
# host side
# --------------------------------------------------------------------------

def _stage_inputs(
    x, embed, adj, W_in, b_in, bn_in_g, bn_in_b, bn_in_rm, bn_in_rv,
    W_gs, b_gs, Wih0, Whh0, bih0, bhh0, Wih1, Whh1, bih1, bhh1,
    W_emb, b_emb, bn_emb_g, bn_emb_b, bn_emb_rm, bn_emb_rv,
    W_fc, b_fc, bn_fc_g, bn_fc_b, bn_fc_rm, bn_fc_rv, W_out, b_out,
):
    x = np.asarray(x, np.float32)
    embed = np.asarray(embed, np.float32)
    adj = np.asarray(adj, np.float32)

    # replicated weight staging
    w_inT = np.zeros((FPAD, NH), ml_dtypes.bfloat16)
    w_inT[:NFEAT] = _bf(np.asarray(W_in, np.float32).T)

    def bn_fold(g, b, rm, rv, lin_b=None):
        g = np.asarray(g, np.float32); b = np.asarray(b, np.float32)
        rm = np.asarray(rm, np.float32); rv = np.asarray(rv, np.float32)
        sc = g / np.sqrt(rv + BN_EPS)
        base = lin_b if lin_b is not None else 0.0
        shv = sc * (base - rm) + b
        return _f32(sc), _f32(shv)

    sc_in, sh_in = bn_fold(bn_in_g, bn_in_b, bn_in_rm, bn_in_rv,
                           np.asarray(b_in, np.float32))
    _, sh_in2 = bn_fold(bn_in_g, bn_in_b, bn_in_rm, bn_in_rv)
    sc_emb, sh_emb = bn_fold(bn_emb_g, bn_emb_b, bn_emb_rm, bn_emb_rv,
                             np.asarray(b_emb, np.float32))
    sc_fc, sh_fc = bn_fold(bn_fc_g, bn_fc_b, bn_fc_rm, bn_fc_rv,
                           np.asarray(b_fc, np.float32))

    W_gs = np.asarray(W_gs, np.float32)
    wgs_sT = _f32(np.stack([W_gs[l][:, :NH].T for l in range(L)]))
    wgs_nT = _f32(np.stack([W_gs[l][:, NH:].T for l in range(L)]))
    bgs = _f32(np.asarray(b_gs, np.float32).T)          # [NH, L]

    wihT = np.stack([_bf(np.asarray(Wih0, np.float32).T),
                     _bf(np.asarray(Wih1, np.float32).T)])
    whhT = np.stack([_bf(np.asarray(Whh0, np.float32).T),
                     _bf(np.asarray(Whh1, np.float32).T)])
    bl = np.stack([np.asarray(bih0, np.float32) + np.asarray(bhh0, np.float32),
                   np.asarray(bih1, np.float32) + np.asarray(bhh1, np.float32)])
    # [512] per layer -> [128, l*4+g]
    blstm = np.zeros((NH, 8), np.float32)
    for l in range(2):
        for g in range(4):
            blstm[:, l * 4 + g] = bl[l][g * NH:(g + 1) * NH]

    w_embT = _f32(np.asarray(W_emb, np.float32).T)
    w_fcT = _f32(np.asarray(W_fc, np.float32).T)
    w_outT = _f32(np.asarray(W_out, np.float32).T)
    bout = _f32(np.tile(np.asarray(b_out, np.float32)[None, :], (128, 1)))

    shared = {
        "w_inT": w_inT,
        "sc_in": sc_in[:, None], "sh_in": sh_in[:, None],
        "sc_in_h": _f32(0.5 * sc_in)[:, None], "sh_in2": sh_in2[:, None],
        "wgs_sT": wgs_sT, "wgs_nT": wgs_nT, "bgs": bgs,
        "wihT": _bf(wihT), "whhT": _bf(whhT), "blstm": blstm,
        "w_embT": w_embT, "sc_emb": sc_emb[:, None], "sh_emb": sh_emb[:, None],
        "w_fcT": w_fcT,
        "sc_fc_a": _f32(sc_fc[:128])[:, None], "sh_fc_a": _f32(sh_fc[:128])[:, None],
        "sc_fc_b": _f32(sc_fc[128:])[:, None], "sh_fc_b": _f32(sh_fc[128:])[:, None],
        "w_outT": w_outT, "bout": bout,
    }

    # adjacency: per-core transposed bf16 shard with padded global ordering
    adj_bf = _bf(adj)
    rowsum = adj.sum(axis=1)                     # fp32, exact rows
    in_maps = []
    for c in range(NC):
        rows = slice(c * NPC, (c + 1) * NPC)
        adjT = np.zeros((NP, PC), ml_dtypes.bfloat16)
        blk = adj_bf[rows].T                     # [10000, 1250] view
        for ck in range(NC):
            adjT[ck * PC:ck * PC + NPC, :NPC] = blk[ck * NPC:(ck + 1) * NPC]
        xT = np.zeros((FPAD, PC), ml_dtypes.bfloat16)
        xT[:NFEAT, :NPC] = _bf(x[rows].T)
        embT = np.zeros((NFE, PC), np.float32)
        embT[:, :NPC] = embed[rows].T
        rec = np.zeros((PC,), np.float32)
        rec[:NPC] = 1.0 / rowsum[rows]
        rsb = np.ascontiguousarray(
            np.broadcast_to(rec[None, :], (128, PC)), dtype=np.float32)
        m = {"adjT": adjT, "xT": xT, "embT": embT, "rsb": rsb}
        m.update(shared)
        in_maps.append(m)
    return in_maps


def kernel(**inputs) -> np.ndarray:
    global _CACHED_NC, LAST_RESULT
    in_maps = _stage_inputs(**inputs)
    if _CACHED_NC is None:
        _CACHED_NC = _build_program()
    nc = _CACHED_NC
    trace = bool(int(os.environ.get("GSAGE_TRACE", "0")))
    res = run_bass_kernel_spmd(
        nc, in_maps, core_ids=list(range(NC)), trace=trace,
    )
    LAST_RESULT = res
    out = np.concatenate(
        [res.results[c]["out"][:NPC] for c in range(NC)], axis=0)
    return np.ascontiguousarray(out, np.float32)


if __name__ == "__main__":
    import reference
    inputs = reference.setup_inputs()
    out = kernel(**{k: np.asarray(v) for k, v in inputs.items()})
    print("out", out.shape, out.dtype)



# revision 15
# speedup vs baseline: 1.5740x; 1.5740x over previous
"""GraphSAGE (gnn_message_passing) forward pass on 8 Trainium2 NeuronCores.

Sharding strategy (hardcoded): row-shard the 10000 nodes across 8 cores
(1250 each, padded to 1280).  The dominant cost is the [10000, 10000]
adjacency: it is quantized to fp8-e4m3 on host (final rel-err ~1e-3 vs
2e-2 tolerance), DMAed once per core as a [10240, 1280] transposed shard
into SBUF where it stays RESIDENT across both GNN layers (layer-1
aggregation does no adj DMA at all).  Node features stay feature-major
on-chip; neighbor features are AllGathered in bf16 (two node-halves per
layer so aggregation on half 0 overlaps the half-1 collective).  The
independent embed projection and the t=0 LSTM cells are scheduled into
the collective-wait windows.  All weights are replicated; all matmuls
run at bf16/fp8 rate (no fp32 PE passes).
"""

import os

import numpy as np
import ml_dtypes

import concourse.bass as bass
import concourse.bacc as bacc
import concourse.mybir as mybir
import concourse.tile as tile
from concourse.bass_utils import run_bass_kernel_spmd
from concourse.masks import make_identity

F32 = mybir.dt.float32
BF16 = mybir.dt.bfloat16
FP8 = mybir.dt.float8e4
AX = mybir.AxisListType
OP = mybir.AluOpType
AF = mybir.ActivationFunctionType

# ---- problem constants (hardcoded per spec) ----
N = 10000        # nodes
NC = 8           # cores
NPC = 1250       # original nodes per core
PC = 1280        # padded nodes per core
NP = NC * PC     # padded total nodes = 10240
KT = NP // 128   # 80 contraction tiles
IT = PC // 128   # 10 node tiles per core
HT = IT // 2     # 5 tiles per gather half
NFEAT = 2000
FPAD = 2048
FT = FPAD // 128  # 16
XGK = 2          # x k-tiles per DMA group
XG = FT // XGK   # 8 x groups
AGK = 8          # adj k-tiles per DMA group
AG = KT // AGK   # 10 adj groups
NH = 128
NHE = 64
NFE = 256
D = NH + NHE     # 192
NOUT = 20
L = 2
BN_EPS = 1e-5

# matmul free-dim chunks over PC (PSUM bank = 512 fp32)
CHUNKS = [(0, 512), (512, 512), (1024, 256)]

# bf16 packed-const column layout
CBF_LAYOUT = [
    ("w_in", FPAD),
    ("wgs_s0", NH), ("wgs_n0", NH), ("wgs_s1", NH), ("wgs_n1", NH),
    ("wih0", 4 * NH), ("whh0", 4 * NH), ("wih1", 4 * NH), ("whh1", 4 * NH),
    ("wemb0", NHE), ("wemb1", NHE),
    ("wfc_a", D), ("wfc_b", D),
    ("wout_a", NOUT), ("wout_b", NOUT),
]
CBF_OFF = {}
_o = 0
for _n, _w in CBF_LAYOUT:
    CBF_OFF[_n] = _o
    _o += _w
CBF_W = _o

# f32 packed-const column layout
CF_LAYOUT = [
    ("sc_in", 1), ("sh_in", 1), ("sc_in_h", 1), ("sh_in2", 1),
    ("bgs0", 1), ("bgs1", 1),
    ("bl00", 1), ("bl01", 1), ("bl02", 1), ("bl03", 1),
    ("bl10", 1), ("bl11", 1), ("bl12", 1), ("bl13", 1),
    ("sc_emb", 1), ("sh_emb", 1),
    ("sc_fc_a", 1), ("sh_fc_a", 1), ("sc_fc_b", 1), ("sh_fc_b", 1),
    ("bout", NOUT),
]
CF_OFF = {}
_o = 0
for _n, _w in CF_LAYOUT:
    CF_OFF[_n] = _o
    _o += _w
CF_W = _o

LAST_RESULT = None  # test.py reads exec_time info from here

_CACHED_NC = None


def _bf(a):
    return np.asarray(a, dtype=ml_dtypes.bfloat16)


def _f8(a):
    return np.asarray(a, dtype=ml_dtypes.float8_e4m3fn)


def _f32(a):
    return np.ascontiguousarray(a, dtype=np.float32)


# --------------------------------------------------------------------------
# device program
# --------------------------------------------------------------------------

def _build_program():
    nc = bacc.Bacc("TRN2", target_bir_lowering=False, debug=False, num_devices=NC)

    def inp(name, shape, dtype):
        return nc.declare_dram_parameter(name, list(shape), dtype, isOutput=False)

    # per-core tensors
    d_adjq = inp("adjq", [AG, 128, AGK * PC], FP8)
    d_xq = inp("xq", [XG, 128, XGK * PC], FP8)
    d_embT = inp("embT", [2, 128, PC], BF16)
    d_rsb = inp("rsb", [128, PC], BF16)       # 1/rowsum broadcast to 128 parts
    # replicated packed weights
    d_cbf = inp("cbf", [128, CBF_W], BF16)
    d_cf = inp("cf", [128, CF_W], F32)
    d_out = nc.declare_dram_parameter("out", [PC, NOUT], F32, isOutput=True)

    # internal DRAM for collectives: per layer, two node-halves
    bounce = [[nc.dram_tensor(f"bounce{l}_{h}", [128, HT * 128], BF16)
               for h in range(2)] for l in range(L)]
    hg = [[nc.dram_tensor(f"hg{l}_{h}", [NC, 128, HT * 128], BF16,
                          addr_space="Shared") for h in range(2)]
          for l in range(L)]
    groups = [list(range(NC))]

    with tile.TileContext(nc) as tc:
        with tc.tile_pool(name="res", bufs=1) as res, \
             tc.tile_pool(name="adjp", bufs=AG) as adjp, \
             tc.tile_pool(name="hnatp", bufs=2) as hnatp, \
             tc.tile_pool(name="xqp", bufs=2) as xqp, \
             tc.tile_pool(name="locp", bufs=2) as locp, \
             tc.tile_pool(name="psBig", bufs=1, space="PSUM") as psBig, \
             tc.tile_pool(name="psSml", bufs=1, space="PSUM") as psSml, \
             tc.tile_pool(name="psT", bufs=1, space="PSUM") as psT, \
             tc.tile_pool(name="psG", bufs=2, space="PSUM") as psG:

            # ---- resident SBUF tensors ----
            adj_gt = []   # per-group adjacency tiles (resident, fp8)
            cbf = res.tile([128, CBF_W], BF16, tag="cbf")
            cf = res.tile([128, CF_W], F32, tag="cf")
            rsb = res.tile([128, PC], BF16, tag="rsb")
            ident_bf = res.tile([128, 128], BF16, tag="ident")
            ones_col = res.tile([128, 1], BF16, tag="ones_col")
            ones_row = res.tile([1, 128], BF16, tag="ones_row")

            h_bf = [res.tile([128, PC], BF16, tag=f"h{i}bf", name=f"h{i}bf")
                    for i in range(3)]
            e_bf = res.tile([64, PC], BF16, tag="e_bf")
            o0_bf = res.tile([128, PC], BF16, tag="o0bf")
            o1_bf = res.tile([128, PC], BF16, tag="o1bf")
            p0_bf = res.tile([128, PC], BF16, tag="p0bf")
            p1_bf = res.tile([128, PC], BF16, tag="p1bf")
            c_st = [res.tile([128, PC], BF16, tag=f"c{i}", name=f"c{i}")
                    for i in range(2)]
            hpost_bf = res.tile([128, PC], BF16, tag="hpostbf")
            hfca_bf = res.tile([128, PC], BF16, tag="hfcabf")
            hfcb_bf = res.tile([64, PC], BF16, tag="hfcbbf")
            outall = res.tile([128, IT * NOUT], F32, tag="outall")
            sem = res.tile([128, IT], F32, tag="sem")
            exs = res.tile([128, NOUT], BF16, tag="exs")
            lse = res.tile([128, IT], F32, tag="lse")
            # scratch (bf16, shared across phases via tags)
            t_y = res.tile([128, PC], BF16, tag="t_y")
            t_e = res.tile([128, PC], BF16, tag="t_e")
            t_r = res.tile([128, PC], BF16, tag="t_r")
            neigh_bf = res.tile([128, PC], BF16, tag="neigh")
            nln = res.tile([1, PC], F32, tag="nln")
            eps1 = res.tile([1, 1], F32, tag="eps1")
            rec_bf = res.tile([1, PC], BF16, tag="rec")
            gact = [res.tile([128, 512], F32, tag=f"ga{g}", name=f"ga{g}")
                    for g in range(4)]
            embs = res.tile([128, 2 * PC], BF16, tag="embs")

            # ---- issue input DMAs ----
            # bulk stream on sync queue; latency-critical on scalar queue
            nc.sync.dma_start(cbf, d_cbf.ap())
            nc.sync.dma_start(cf, d_cf.ap())
            xq_tiles = []
            for g in range(XG):
                xt = xqp.tile([128, XGK * PC], FP8, tag="xq", name=f"xq{g}")
                nc.sync.dma_start(xt, d_xq[g])
                xq_tiles.append(xt)
            for g in range(AG):
                at = adjp.tile([128, AGK * PC], FP8, tag="adjg", name=f"adj{g}")
                nc.sync.dma_start(at, d_adjq[g])
                adj_gt.append(at)
            nc.scalar.dma_start(rsb, d_rsb.ap())
            nc.scalar.dma_start(
                embs.rearrange("p (t i) -> p t i", t=2),
                d_embT.ap().rearrange("t p i -> p t i"))

            make_identity(nc, ident_bf)
            nc.vector.memset(ones_col, 1.0)
            nc.vector.memset(ones_row, 1.0)
            nc.vector.memset(eps1, 1e-24)

            def cfv(nm):
                return cf[:, CF_OFF[nm]:CF_OFF[nm] + 1]

            def cfv64(nm):
                return cf[:64, CF_OFF[nm]:CF_OFF[nm] + 1]

            def wbf(nm, p=128):
                w = dict(CBF_LAYOUT)[nm]
                return cbf[:p, CBF_OFF[nm]:CBF_OFF[nm] + w]

            # ---- helpers ----
            def mm_acc(psum_ap, lhsT, rhs, start, stop):
                F = rhs.shape[-1]
                o = 0
                while o < F:
                    w = min(512, F - o)
                    nc.tensor.matmul(
                        psum_ap[:, o:o + w], lhsT, rhs[:, o:o + w],
                        start=start, stop=stop,
                    )
                    o += w

            def elu_from(out_sb, in_ap, sc_ap, sh_ap):
                """out = elu(sc*in + sh); in_ap may be PSUM; [P, F] bf16 out"""
                P, F = out_sb.shape[0], out_sb.shape[-1]
                y = t_y[:P, :F]
                e = t_e[:P, :F]
                nc.vector.tensor_scalar(y, in_ap, sc_ap, sh_ap, OP.mult, OP.add)
                nc.vector.tensor_scalar_min(e, y, 0.0)
                nc.scalar.activation(e, e, AF.Exp)
                nc.vector.tensor_scalar(y, y, 0.0, -1.0, OP.max, OP.add)
                nc.vector.tensor_tensor(out_sb, y, e, OP.add)

            def send_half(l, h, src_bf):
                """transpose node-half h of src_bf, DMA to bounce, AllGather"""
                loc = locp.tile([128, HT * 128], BF16, tag="loc",
                                name=f"loc{l}_{h}")
                for s in range(HT):
                    it = h * HT + s
                    pt = psT.tile([128, 128], BF16, tag="tp", name="tp")
                    nc.tensor.transpose(
                        pt, src_bf[:, it * 128:(it + 1) * 128], ident_bf)
                    nc.vector.tensor_copy(loc[:, s * 128:(s + 1) * 128], pt)
                nc.scalar.dma_start(bounce[l][h].ap(), loc)
                nc.gpsimd.collective_compute(
                    "AllGather", OP.bypass, replica_groups=groups,
                    ins=[bounce[l][h].ap().opt()],
                    outs=[hg[l][h].ap().opt()],
                )

            def recv_half(l, h):
                hnat = hnatp.tile([128, NC * HT * 128], BF16, tag="hnat",
                                  name=f"hnat{l}_{h}")
                nc.scalar.dma_start(
                    hnat.rearrange("p (c v) -> p c v", c=NC),
                    hg[l][h].ap().rearrange("c p v -> p c v"))
                return hnat

            def agg_half(ps, hnat, h, start):
                """accumulate half-h k-tiles of the adjacency into ps"""
                for c in range(NC):
                    for s in range(HT):
                        kt = c * IT + h * HT + s
                        lhsT = hnat[:, (c * HT + s) * 128:(c * HT + s + 1) * 128]
                        first = start and c == 0 and s == 0
                        last = (not start) and c == NC - 1 and s == HT - 1
                        rhs = adj_gt[kt // AGK][
                            :, (kt % AGK) * PC:(kt % AGK + 1) * PC]
                        mm_acc(ps, lhsT, rhs, start=first, stop=last)

            def norm_cols(dst_bf, hrelu):
                """dst = hrelu / ||hrelu||_col  (column L2 norm over 128 feats)"""
                sq = t_y  # scratch
                nc.vector.tensor_tensor(sq, hrelu, hrelu, OP.mult)
                for (o, w) in CHUNKS:
                    ps_ss = psSml.tile([1, 512], F32, tag="ss", name="ps_ss")
                    nc.tensor.matmul(ps_ss[:, :w], ones_col, sq[:, o:o + w],
                                     start=True, stop=True)
                    nc.scalar.activation(nln[:, o:o + w], ps_ss[:, :w], AF.Ln,
                                         bias=eps1)
                # 1/sqrt(n2) = exp(-0.5 * ln(n2))
                nc.scalar.activation(rec_bf, nln, AF.Exp, scale=-0.5)
                for (o, w) in CHUNKS:
                    ps_bc = psSml.tile([128, 512], F32, tag="bc", name="ps_bc")
                    nc.tensor.matmul(ps_bc[:, :w], ones_row, rec_bf[:, o:o + w],
                                     start=True, stop=True)
                    nc.vector.tensor_tensor(dst_bf[:, o:o + w],
                                            hrelu[:, o:o + w], ps_bc[:, :w],
                                            OP.mult)

            def lstm_cell(l, t, xin_bf, hprev_bf, c_tile, out_bf):
                """one LSTM cell; t==0 skips the f gate (c_prev == 0)"""
                wih = wbf(f"wih{l}")
                whh = wbf(f"whh{l}")
                for (o, w) in CHUNKS:
                    # gate order: sigmoid batch (i, f, o) then tanh (g)
                    glist = [0, 1, 3, 2] if t > 0 else [0, 3, 2]
                    gps = {}
                    for g in glist:
                        ps = psG.tile([128, 512], F32, tag="gate",
                                      name=f"g{g}")
                        nc.tensor.matmul(
                            ps[:, :w], wih[:, g * 128:(g + 1) * 128],
                            xin_bf[:, o:o + w], start=True, stop=(t == 0))
                        if t > 0:
                            nc.tensor.matmul(
                                ps[:, :w], whh[:, g * 128:(g + 1) * 128],
                                hprev_bf[:, o:o + w], start=False, stop=True)
                        gps[g] = ps
                    ga = {}
                    for g in glist:
                        fn = AF.Tanh if g == 2 else AF.Sigmoid
                        gt = gact[g][:, :w]
                        nc.scalar.activation(gt, gps[g][:, :w], fn,
                                             bias=cfv(f"bl{l}{g}"))
                        ga[g] = gt
                    cs = c_tile[:, o:o + w]
                    if t == 0:
                        nc.vector.tensor_tensor(cs, ga[0], ga[2], OP.mult)
                    else:
                        fc_ = t_y[:, o:o + w]
                        nc.vector.tensor_tensor(fc_, ga[1], cs, OP.mult)
                        igg = t_e[:, o:o + w]
                        nc.vector.tensor_tensor(igg, ga[0], ga[2], OP.mult)
                        nc.vector.tensor_tensor(cs, fc_, igg, OP.add)
                    tc_ = gact[2][:, :w]
                    nc.scalar.activation(tc_, cs, AF.Tanh)
                    nc.vector.tensor_tensor(out_bf[:, o:o + w], ga[3], tc_,
                                            OP.mult)

            # ================= pipeline =================

            # ---- input projection: h0 = elu(bn(W_in @ x)) ----
            ps = psBig.tile([128, PC], F32, tag="big", name="ps_proj")
            w_in = wbf("w_in")
            for g in range(XG):
                for j in range(XGK):
                    t = g * XGK + j
                    mm_acc(ps, w_in[:, t * 128:(t + 1) * 128],
                           xq_tiles[g][:, j * PC:(j + 1) * PC],
                           start=(t == 0), stop=(t == FT - 1))
            elu_from(h_bf[0], ps, cfv("sc_in"), cfv("sh_in"))

            # ---- gather h0 (two node-halves) ----
            send_half(0, 0, h_bf[0])
            send_half(0, 1, h_bf[0])

            # ---- embed projection in the collective window ----
            ps_e = psBig.tile([128, PC], F32, tag="big", name="ps_e")
            for ti in range(2):
                mm_acc(ps_e[:64, :], wbf(f"wemb{ti}"),
                       embs[:, ti * PC:(ti + 1) * PC],
                       start=(ti == 0), stop=(ti == 1))
            elu_from(e_bf, ps_e[:64, :], cfv64("sc_emb"), cfv64("sh_emb"))

            # ---- GNN layers ----
            for l in range(L):
                ps_agg = psBig.tile([128, PC], F32, tag="big", name="ps_agg")
                hnat0 = recv_half(l, 0)
                agg_half(ps_agg, hnat0, 0, start=True)
                hnat1 = recv_half(l, 1)
                agg_half(ps_agg, hnat1, 1, start=False)
                nc.vector.tensor_tensor(neigh_bf, ps_agg, rsb, OP.mult)

                # GS linear: relu(W_self @ h + W_neigh @ neigh + b)
                ps_gs = psBig.tile([128, PC], F32, tag="big", name="ps_gs")
                mm_acc(ps_gs, wbf(f"wgs_s{l}"), h_bf[l], start=True, stop=False)
                mm_acc(ps_gs, wbf(f"wgs_n{l}"), neigh_bf, start=False, stop=True)
                hrelu = t_r
                nc.scalar.activation(hrelu, ps_gs, AF.Relu,
                                     bias=cfv(f"bgs{l}"), scale=1.0)
                norm_cols(h_bf[l + 1], hrelu)

                if l == 0:
                    # send h1 for the next layer, then fill the collective
                    # window with the t=0 LSTM cells
                    send_half(1, 0, h_bf[1])
                    send_half(1, 1, h_bf[1])
                    lstm_cell(0, 0, h_bf[1], None, c_st[0], o0_bf)
                    lstm_cell(1, 0, o0_bf, None, c_st[1], p0_bf)

            # ---- remaining LSTM cells ----
            lstm_cell(0, 1, h_bf[2], o0_bf, c_st[0], o1_bf)
            lstm_cell(1, 1, o1_bf, p0_bf, c_st[1], p1_bf)

            # ---- post: JK mean -> bn/elu ; fc ; logits ; log_softmax ----
            hsum = t_r
            nc.vector.tensor_tensor(hsum, p0_bf, p1_bf, OP.add)
            elu_from(hpost_bf, hsum, cfv("sc_in_h"), cfv("sh_in2"))

            ps_fa = psBig.tile([128, PC], F32, tag="big", name="ps_fa")
            mm_acc(ps_fa, wbf("wfc_a")[:, :128], hpost_bf, start=True, stop=False)
            mm_acc(ps_fa, wbf("wfc_b", 64)[:, :128], e_bf, start=False, stop=True)
            elu_from(hfca_bf, ps_fa, cfv("sc_fc_a"), cfv("sh_fc_a"))
            ps_fb = psBig.tile([128, PC], F32, tag="big", name="ps_fb")
            mm_acc(ps_fb[:64, :], wbf("wfc_a")[:, 128:], hpost_bf,
                   start=True, stop=False)
            mm_acc(ps_fb[:64, :], wbf("wfc_b", 64)[:, 128:], e_bf,
                   start=False, stop=True)
            elu_from(hfcb_bf, ps_fb[:64, :], cfv64("sc_fc_b"), cfv64("sh_fc_b"))

            # logits per node-tile; log_softmax without max-subtraction
            bout = cf[:, CF_OFF["bout"]:CF_OFF["bout"] + NOUT]
            for it in range(IT):
                ps_lg = psSml.tile([128, 512], F32, tag="bc",
                                   name="ps_lg")[:, :NOUT]
                nc.tensor.matmul(ps_lg, hfca_bf[:, it * 128:(it + 1) * 128],
                                 wbf("wout_a"), start=True, stop=False)
                nc.tensor.matmul(ps_lg, hfcb_bf[:, it * 128:(it + 1) * 128],
                                 wbf("wout_b", 64), start=False, stop=True)
                nc.vector.tensor_tensor(
                    outall[:, it * NOUT:(it + 1) * NOUT], ps_lg, bout, OP.add)
            for it in range(IT):
                nc.scalar.activation(
                    exs, outall[:, it * NOUT:(it + 1) * NOUT], AF.Exp,
                    accum_out=sem[:, it:it + 1])
            nc.scalar.activation(lse, sem, AF.Ln)
            for it in range(IT):
                sl = outall[:, it * NOUT:(it + 1) * NOUT]
                nc.vector.tensor_scalar(sl, sl, lse[:, it:it + 1], None,
                                        OP.subtract)

            nc.scalar.dma_start(
                d_out.ap().rearrange("(t p) c -> p t c", p=128),
                outall.rearrange("p (t c) -> p t c", t=IT))

    nc.compile()
    return nc


# --------------------------------------------------------------------------
# host side
# --------------------------------------------------------------------------

def _stage_inputs(
    x, embed, adj, W_in, b_in, bn_in_g, bn_in_b, bn_in_rm, bn_in_rv,
    W_gs, b_gs, Wih0, Whh0, bih0, bhh0, Wih1, Whh1, bih1, bhh1,
    W_emb, b_emb, bn_emb_g, bn_emb_b, bn_emb_rm, bn_emb_rv,
    W_fc, b_fc, bn_fc_g, bn_fc_b, bn_fc_rm, bn_fc_rv, W_out, b_out,
):
    x = np.asarray(x, np.float32)
    embed = np.asarray(embed, np.float32)
    adj = np.asarray(adj, np.float32)

    def bn_fold(g, b, rm, rv, lin_b=None):
        g = np.asarray(g, np.float32); b = np.asarray(b, np.float32)
        rm = np.asarray(rm, np.float32); rv = np.asarray(rv, np.float32)
        sc = g / np.sqrt(rv + BN_EPS)
        base = lin_b if lin_b is not None else 0.0
        shv = sc * (base - rm) + b
        return _f32(sc), _f32(shv)

    sc_in, sh_in = bn_fold(bn_in_g, bn_in_b, bn_in_rm, bn_in_rv,
                           np.asarray(b_in, np.float32))
    _, sh_in2 = bn_fold(bn_in_g, bn_in_b, bn_in_rm, bn_in_rv)
    sc_emb, sh_emb = bn_fold(bn_emb_g, bn_emb_b, bn_emb_rm, bn_emb_rv,
                             np.asarray(b_emb, np.float32))
    sc_fc, sh_fc = bn_fold(bn_fc_g, bn_fc_b, bn_fc_rm, bn_fc_rv,
                           np.asarray(b_fc, np.float32))

    # ---- packed bf16 consts ----
    cbf = np.zeros((128, CBF_W), ml_dtypes.bfloat16)

    def put(nm, arr):
        arr = np.asarray(arr, np.float32)
        p, w = arr.shape
        cbf[:p, CBF_OFF[nm]:CBF_OFF[nm] + w] = _bf(arr)

    W_in = np.asarray(W_in, np.float32)
    w_inT = np.zeros((FPAD, NH), np.float32)
    w_inT[:NFEAT] = W_in.T
    # w_in sbuf layout: [p, t*128 + j] = W_inT[t*128 + p, j]
    put("w_in", w_inT.reshape(FT, 128, NH).transpose(1, 0, 2).reshape(128, FPAD))

    W_gs = np.asarray(W_gs, np.float32)
    for l in range(L):
        put(f"wgs_s{l}", W_gs[l][:, :NH].T)
        put(f"wgs_n{l}", W_gs[l][:, NH:].T)
    put("wih0", np.asarray(Wih0, np.float32).T)
    put("whh0", np.asarray(Whh0, np.float32).T)
    put("wih1", np.asarray(Wih1, np.float32).T)
    put("whh1", np.asarray(Whh1, np.float32).T)
    W_emb = np.asarray(W_emb, np.float32)
    put("wemb0", W_emb[:, :128].T)
    put("wemb1", W_emb[:, 128:].T)
    W_fc = np.asarray(W_fc, np.float32)
    put("wfc_a", W_fc[:, :128].T)      # [128, 192]
    put("wfc_b", W_fc[:, 128:].T)      # [64, 192]
    W_out = np.asarray(W_out, np.float32)
    put("wout_a", W_out[:, :128].T)
    put("wout_b", W_out[:, 128:].T)

    # ---- packed f32 consts ----
    cfp = np.zeros((128, CF_W), np.float32)

    def putf(nm, vec, p=128):
        v = np.asarray(vec, np.float32).reshape(-1)
        cfp[:p, CF_OFF[nm]] = v

    putf("sc_in", sc_in); putf("sh_in", sh_in)
    putf("sc_in_h", 0.5 * sc_in); putf("sh_in2", sh_in2)
    b_gs = np.asarray(b_gs, np.float32)
    putf("bgs0", b_gs[0]); putf("bgs1", b_gs[1])
    bl0 = np.asarray(bih0, np.float32) + np.asarray(bhh0, np.float32)
    bl1 = np.asarray(bih1, np.float32) + np.asarray(bhh1, np.float32)
    for g in range(4):
        putf(f"bl0{g}", bl0[g * NH:(g + 1) * NH])
        putf(f"bl1{g}", bl1[g * NH:(g + 1) * NH])
    putf("sc_emb", sc_emb, 64); putf("sh_emb", sh_emb, 64)
    putf("sc_fc_a", sc_fc[:128]); putf("sh_fc_a", sh_fc[:128])
    putf("sc_fc_b", sc_fc[128:], 64); putf("sh_fc_b", sh_fc[128:], 64)
    cfp[:, CF_OFF["bout"]:CF_OFF["bout"] + NOUT] = np.asarray(
        b_out, np.float32)[None, :]

    shared = {"cbf": cbf, "cf": cfp}

    rowsum = adj.sum(axis=1)                     # fp32, exact rows
    in_maps = []
    for c in range(NC):
        rows = slice(c * NPC, (c + 1) * NPC)
        # transposed fp8 adjacency shard with padded global node ordering
        adjT = np.zeros((NP, PC), ml_dtypes.float8_e4m3fn)
        blk = _f8(adj[rows].T)                   # [10000, 1250]
        for ck in range(NC):
            adjT[ck * PC:ck * PC + NPC, :NPC] = blk[ck * NPC:(ck + 1) * NPC]
        # group-major DMA layout: [g, p, tt*PC + i]
        adjq = np.ascontiguousarray(
            adjT.reshape(AG, AGK, 128, PC).transpose(0, 2, 1, 3)
            .reshape(AG, 128, AGK * PC))

        xT = np.zeros((FPAD, PC), ml_dtypes.float8_e4m3fn)
        xT[:NFEAT, :NPC] = _f8(x[rows].T)
        xq = np.ascontiguousarray(
            xT.reshape(XG, XGK, 128, PC).transpose(0, 2, 1, 3)
            .reshape(XG, 128, XGK * PC))

        embT = np.zeros((2, 128, PC), ml_dtypes.bfloat16)
        embT[:, :, :NPC] = _bf(embed[rows].T.reshape(2, 128, NPC))

        rec = np.zeros((PC,), np.float32)
        rec[:NPC] = 1.0 / rowsum[rows]
        rsb = np.ascontiguousarray(
            np.broadcast_to(_bf(rec)[None, :], (128, PC)))

        m = {"adjq": adjq, "xq": xq, "embT": embT, "rsb": rsb}
        m.update(shared)
        in_maps.append(m)
    return in_maps


def kernel(**inputs) -> np.ndarray:
    global _CACHED_NC, LAST_RESULT
    in_maps = _stage_inputs(**inputs)
    if _CACHED_NC is None:
        _CACHED_NC = _build_program()
    nc = _CACHED_NC
    trace = bool(int(os.environ.get("GSAGE_TRACE", "0")))
    res = run_bass_kernel_spmd(
        nc, in_maps, core_ids=list(range(NC)), trace=trace,
    )
    LAST_RESULT = res
    out = np.concatenate(
        [res.results[c]["out"][:NPC] for c in range(NC)], axis=0)
    return np.ascontiguousarray(out, np.float32)


if __name__ == "__main__":
    import reference
    inputs = reference.setup_inputs()
    out = kernel(**{k: np.asarray(v) for k, v in inputs.items()})
    print("out", out.shape, out.dtype)


# revision 33
# speedup vs baseline: 1.9944x; 1.2671x over previous
"""GraphSAGE (gnn_message_passing) forward pass on 8 Trainium2 NeuronCores.

Sharding strategy (hardcoded): row-shard the 10000 nodes across 8 cores
(1250 each, padded to 1280).  The dominant cost is the [10000, 10000]
adjacency: it is quantized to fp8-e4m3 on host (final rel-err ~1e-3 vs
2e-2 tolerance), DMAed once per core as a [10240, 1280] transposed shard
into SBUF where it stays RESIDENT across both GNN layers (layer-1
aggregation does no adj DMA at all).  Node features stay feature-major
on-chip; neighbor features are AllGathered in bf16 (two node-halves per
layer so aggregation on half 0 overlaps the half-1 collective).  The
independent embed projection and the t=0 LSTM cells are scheduled into
the collective-wait windows.  All weights are replicated; all matmuls
run at bf16/fp8 rate (no fp32 PE passes).
"""

import os
from contextlib import ExitStack

import numpy as np
import ml_dtypes

import concourse.bass as bass
import concourse.bacc as bacc
import concourse.mybir as mybir
import concourse.tile as tile
from concourse.bass_utils import run_bass_kernel_spmd
from concourse.masks import make_identity

F32 = mybir.dt.float32
BF16 = mybir.dt.bfloat16
FP8 = mybir.dt.float8e4
AX = mybir.AxisListType
OP = mybir.AluOpType
AF = mybir.ActivationFunctionType

# ---- problem constants (hardcoded per spec) ----
N = 10000        # nodes
NC = 8           # cores
NPC = 1250       # original nodes per core
PC = 1280        # padded nodes per core
NP = NC * PC     # padded total nodes = 10240
KT = NP // 128   # 80 contraction tiles
IT = PC // 128   # 10 node tiles per core
HT = IT // 2     # 5 tiles per gather half
NFEAT = 2000
FPAD = 2048
FT = FPAD // 128  # 16
XGK = 2          # x k-tiles per DMA group
XG = FT // XGK   # 8 x groups
AGK = 10         # adj k-tiles per DMA group (= one core's k-tiles)
AG = KT // AGK   # 8 adj groups
NH = 128
NHE = 64
NFE = 256
D = NH + NHE     # 192
NOUT = 20
L = 2
BN_EPS = 1e-5

# matmul free-dim chunks over PC (PSUM bank = 512 fp32)
CHUNKS = [(0, 512), (512, 512), (1024, 256)]

# bf16 packed-const column layout
CBF_LAYOUT = [
    ("w_in", FPAD),
    ("wgs_s0", NH), ("wgs_n0", NH), ("wgs_s1", NH), ("wgs_n1", NH),
    ("wih0", 4 * NH), ("whh0", 4 * NH), ("wih1", 4 * NH), ("whh1", 4 * NH),
    ("wemb0", NHE), ("wemb1", NHE),
    ("wfc_a", D), ("wfc_b", D),
    ("wout_a", NOUT), ("wout_b", NOUT),
]
CBF_OFF = {}
_o = 0
for _n, _w in CBF_LAYOUT:
    CBF_OFF[_n] = _o
    _o += _w
CBF_W = _o

# f32 packed-const column layout
CF_LAYOUT = [
    ("sc_in", 1), ("sh_in", 1), ("sc_in_h", 1), ("sh_in2", 1),
    ("bgs0", 1), ("bgs1", 1),
    ("bl00", 1), ("bl01", 1), ("bl02", 1), ("bl03", 1),
    ("bl10", 1), ("bl11", 1), ("bl12", 1), ("bl13", 1),
    ("sc_emb", 1), ("sh_emb", 1),
    ("sc_fc_a", 1), ("sh_fc_a", 1), ("sc_fc_b", 1), ("sh_fc_b", 1),
    ("bout", NOUT),
]
CF_OFF = {}
_o = 0
for _n, _w in CF_LAYOUT:
    CF_OFF[_n] = _o
    _o += _w
CF_W = _o

LAST_RESULT = None  # test.py reads exec_time info from here

_CACHED_NC = None


def _bf(a):
    return np.asarray(a, dtype=ml_dtypes.bfloat16)


def _f8(a):
    return np.asarray(a, dtype=ml_dtypes.float8_e4m3fn)


def _f32(a):
    return np.ascontiguousarray(a, dtype=np.float32)


# --------------------------------------------------------------------------
# device program
# --------------------------------------------------------------------------

def _build_program():
    nc = bacc.Bacc("TRN2", target_bir_lowering=False, debug=False, num_devices=NC)

    def inp(name, shape, dtype):
        return nc.declare_dram_parameter(name, list(shape), dtype, isOutput=False)

    # per-core tensors
    d_adjq = inp("adjq", [AG, 128, AGK * PC], FP8)
    d_xq = inp("xq", [XG, 128, XGK * PC], FP8)
    d_embT = inp("embT", [2, 128, PC], BF16)
    d_rsb = inp("rsb", [128, PC], BF16)       # 1/rowsum broadcast to 128 parts
    # replicated packed weights
    d_cbf = inp("cbf", [128, CBF_W], BF16)
    d_cf = inp("cf", [128, CF_W], F32)
    d_out = nc.declare_dram_parameter("out", [PC, NOUT], F32, isOutput=True)

    # internal DRAM for collectives: per layer, two node-halves (fp8)
    bounce = [[nc.dram_tensor(f"bounce{l}_{h}", [128, HT * 128], FP8)
               for h in range(2)] for l in range(L)]
    hg = [[nc.dram_tensor(f"hg{l}_{h}", [NC, 128, HT * 128], FP8,
                          addr_space="Shared") for h in range(2)]
          for l in range(L)]
    warm_in = nc.dram_tensor("warm_in", [1, 128], BF16)
    warm_out = nc.dram_tensor("warm_out", [NC, 128], BF16,
                              addr_space="Shared")
    groups = [list(range(NC))]

    with tile.TileContext(nc) as tc:
        with tc.tile_pool(name="res", bufs=1) as res, \
             tc.tile_pool(name="adjp", bufs=AG) as adjp, \
             tc.tile_pool(name="hnatp", bufs=2) as hnatp, \
             tc.tile_pool(name="locp", bufs=2) as locp, \
             tc.tile_pool(name="psBig", bufs=1, space="PSUM") as psBig, \
             tc.tile_pool(name="psSml", bufs=1, space="PSUM") as psSml, \
             tc.tile_pool(name="psT", bufs=1, space="PSUM") as psT, \
             tc.tile_pool(name="psG", bufs=2, space="PSUM") as psG:

            # ---- resident SBUF tensors ----
            adj_gt = []   # per-group adjacency tiles (resident, fp8)
            cbf = res.tile([128, CBF_W], BF16, tag="cbf")
            cf = res.tile([128, CF_W], F32, tag="cf")
            rsb = res.tile([128, PC], BF16, tag="rsb")
            ident_bf = res.tile([128, 128], BF16, tag="ident")
            ones_col = res.tile([128, 1], BF16, tag="ones_col")
            ones_row = res.tile([1, 128], BF16, tag="ones_row")

            h_bf = [res.tile([128, PC], BF16, tag=f"h{i}bf", name=f"h{i}bf")
                    for i in range(3)]
            e_bf = res.tile([64, PC], BF16, tag="e_bf")
            o0_bf = res.tile([128, PC], BF16, tag="o0bf")
            o1_bf = res.tile([128, PC], BF16, tag="o1bf")
            p0_bf = res.tile([128, PC], BF16, tag="p0bf")
            p1_bf = res.tile([128, PC], BF16, tag="p1bf")
            c_st = [res.tile([128, PC], BF16, tag=f"c{i}", name=f"c{i}")
                    for i in range(2)]
            hpost_bf = res.tile([128, PC], BF16, tag="hpostbf")
            hfca_bf = res.tile([128, PC], BF16, tag="hfcabf")
            hfcb_bf = res.tile([64, PC], BF16, tag="hfcbbf")
            outall = res.tile([128, IT * NOUT], F32, tag="outall")
            sem = res.tile([128, IT], F32, tag="sem")
            exs = res.tile([128, NOUT], BF16, tag="exs")
            lse = res.tile([128, IT], F32, tag="lse")
            # scratch (bf16, shared across phases via tags)
            t_y = res.tile([128, PC], BF16, tag="t_y")
            t_e = res.tile([128, PC], BF16, tag="t_e")
            t_r = res.tile([128, PC], BF16, tag="t_r")
            neigh_bf = res.tile([128, PC], BF16, tag="neigh")
            nln = res.tile([1, PC], F32, tag="nln")
            eps1 = res.tile([1, 1], F32, tag="eps1")
            rec_bf = res.tile([1, PC], BF16, tag="rec")
            gact = [res.tile([128, 512], BF16, tag=f"ga{g}", name=f"ga{g}")
                    for g in range(4)]

            # warm up the CC ring so the first real AllGather is not slow
            nc.gpsimd.collective_compute(
                "AllGather", OP.bypass, replica_groups=groups,
                ins=[warm_in.ap().opt()], outs=[warm_out.ap().opt()],
            )

            # ---- issue input DMAs ----
            # bulk stream on sync queue; latency-critical on scalar queue
            pa_stack = ExitStack()
            pA = pa_stack.enter_context(tc.tile_pool(name="pA", bufs=2))
            embs = pA.tile([128, 2 * PC], BF16, tag="embs")
            nc.sync.dma_start(cbf, d_cbf.ap())
            nc.sync.dma_start(cf, d_cf.ap())
            xq_tiles = []
            for g in range(XG):
                xt = pA.tile([128, XGK * PC], FP8, tag="xq", bufs=2,
                             name=f"xq{g}")
                nc.sync.dma_start(xt, d_xq[g])
                xq_tiles.append(xt)
            for g in range(AG):
                at = adjp.tile([128, AGK * PC], FP8, tag="adjg", name=f"adj{g}")
                nc.sync.dma_start(at, d_adjq[g])
                adj_gt.append(at)
            nc.scalar.dma_start(rsb, d_rsb.ap())
            nc.scalar.dma_start(
                embs.rearrange("p (t i) -> p t i", t=2),
                d_embT.ap().rearrange("t p i -> p t i"))

            make_identity(nc, ident_bf)
            nc.vector.memset(ones_col, 1.0)
            nc.vector.memset(ones_row, 1.0)
            nc.vector.memset(eps1, 1e-24)

            def cfv(nm):
                return cf[:, CF_OFF[nm]:CF_OFF[nm] + 1]

            def cfv64(nm):
                return cf[:64, CF_OFF[nm]:CF_OFF[nm] + 1]

            def wbf(nm, p=128):
                w = dict(CBF_LAYOUT)[nm]
                return cbf[:p, CBF_OFF[nm]:CBF_OFF[nm] + w]

            # ---- helpers ----
            def mm_acc(psum_ap, lhsT, rhs, start, stop):
                F = rhs.shape[-1]
                o = 0
                while o < F:
                    w = min(512, F - o)
                    nc.tensor.matmul(
                        psum_ap[:, o:o + w], lhsT, rhs[:, o:o + w],
                        start=start, stop=stop,
                    )
                    o += w

            def elu_from(out_sb, in_ap, sc_ap, sh_ap):
                """out = elu(sc*in + sh); in_ap may be PSUM; [P, F] bf16 out"""
                P, F = out_sb.shape[0], out_sb.shape[-1]
                y = t_y[:P, :F]
                e = t_e[:P, :F]
                nc.vector.tensor_scalar(y, in_ap, sc_ap, sh_ap, OP.mult, OP.add)
                nc.vector.tensor_scalar_min(e, y, 0.0)
                nc.scalar.activation(e, e, AF.Exp)
                nc.vector.tensor_scalar(y, y, 0.0, -1.0, OP.max, OP.add)
                nc.vector.tensor_tensor(out_sb, y, e, OP.add)

            def send_half(l, h, src_bf):
                """transpose node-half h of src_bf, AllGather it as fp8"""
                loc = locp.tile([128, HT * 128], FP8, tag="loc",
                                name=f"loc{l}_{h}")
                for s in range(HT):
                    it = h * HT + s
                    pt = psT.tile([128, 128], BF16, tag="tp", name="tp")
                    nc.tensor.transpose(
                        pt, src_bf[:, it * 128:(it + 1) * 128], ident_bf)
                    nc.vector.tensor_copy(loc[:, s * 128:(s + 1) * 128], pt)
                nc.scalar.dma_start(bounce[l][h].ap(), loc)
                nc.gpsimd.collective_compute(
                    "AllGather", OP.bypass, replica_groups=groups,
                    ins=[bounce[l][h].ap().opt()],
                    outs=[hg[l][h].ap().opt()],
                )

            def recv_half(l, h):
                hnat = hnatp.tile([128, NC * HT * 128], FP8, tag="hnat",
                                  name=f"hnat{l}_{h}")
                nc.scalar.dma_start(
                    hnat.rearrange("p (c v) -> p c v", c=NC),
                    hg[l][h].ap().rearrange("c p v -> p c v"))
                return hnat

            def agg_half(ps, hnat, h, start):
                """accumulate half-h k-tiles of the adjacency into ps.

                k-tiles are paired for fp8 DoubleRow (2 MACs/cell/cycle);
                the odd 5th tile of each core-half runs as a normal matmul.
                """
                off = h * HT
                for c in range(NC):
                    grp = adj_gt[c]      # group c holds k-tiles c*10..c*10+9
                    for pr in range(2):
                        s0 = pr * 2
                        lhsT = hnat[:, (c * HT + s0) * 128:
                                    (c * HT + s0 + 2) * 128].rearrange(
                            "p (k f) -> p k f", k=2)
                        rhs = grp[:, (off + s0) * PC:(off + s0 + 2) * PC
                                  ].rearrange("p (k i) -> p k i", k=2)
                        first = start and c == 0 and pr == 0
                        for (o, w) in CHUNKS:
                            nc.tensor.matmul(
                                ps[:, o:o + w], lhsT, rhs[:, :, o:o + w],
                                start=first, stop=False,
                                perf_mode=mybir.MatmulPerfMode.DoubleRow)
                    lhsT1 = hnat[:, (c * HT + 4) * 128:(c * HT + 5) * 128]
                    last = (not start) and c == NC - 1
                    mm_acc(ps, lhsT1, grp[:, (off + 4) * PC:(off + 5) * PC],
                           start=False, stop=last)

            def norm_cols(dst_bf, hrelu):
                """dst = hrelu / ||hrelu||_col  (column L2 norm over 128 feats)"""
                sq = t_y  # scratch
                nc.vector.tensor_tensor(sq, hrelu, hrelu, OP.mult)
                for (o, w) in CHUNKS:
                    ps_ss = psSml.tile([1, 512], F32, tag="ss", name="ps_ss")
                    nc.tensor.matmul(ps_ss[:, :w], ones_col, sq[:, o:o + w],
                                     start=True, stop=True)
                    nc.scalar.activation(nln[:, o:o + w], ps_ss[:, :w], AF.Ln,
                                         bias=eps1)
                # 1/sqrt(n2) = exp(-0.5 * ln(n2))
                nc.scalar.activation(rec_bf, nln, AF.Exp, scale=-0.5)
                for (o, w) in CHUNKS:
                    ps_bc = psSml.tile([128, 512], F32, tag="bc", name="ps_bc")
                    nc.tensor.matmul(ps_bc[:, :w], ones_row, rec_bf[:, o:o + w],
                                     start=True, stop=True)
                    nc.vector.tensor_tensor(dst_bf[:, o:o + w],
                                            hrelu[:, o:o + w], ps_bc[:, :w],
                                            OP.mult)

            def lstm_cell(l, t, xin_bf, hprev_bf, c_tile, out_bf, zhh=None):
                """one LSTM cell; t==0 skips the f gate (c_prev == 0).

                zhh: optional precomputed Whh @ h_prev in SBUF ([128, 4*PC]
                bf16, gate-major) — removes the hh matmuls from the
                critical path.
                """
                wih = wbf(f"wih{l}")
                whh = wbf(f"whh{l}")
                for (o, w) in CHUNKS:
                    # gate order: sigmoid batch (i, f, o) then tanh (g)
                    glist = [0, 1, 3, 2] if t > 0 else [0, 3, 2]
                    gps = {}
                    for g in glist:
                        ps = psG.tile([128, 512], F32, tag="gate",
                                      name=f"g{g}")
                        nc.tensor.matmul(
                            ps[:, :w], wih[:, g * 128:(g + 1) * 128],
                            xin_bf[:, o:o + w], start=True,
                            stop=(t == 0 or zhh is not None))
                        if t > 0 and zhh is None:
                            nc.tensor.matmul(
                                ps[:, :w], whh[:, g * 128:(g + 1) * 128],
                                hprev_bf[:, o:o + w], start=False, stop=True)
                        gps[g] = ps
                    ga = {}
                    for g in glist:
                        fn = AF.Tanh if g == 2 else AF.Sigmoid
                        gt = gact[g][:, :w]
                        if zhh is not None:
                            nc.vector.tensor_tensor(
                                gt, gps[g][:, :w],
                                zhh[:, g * PC + o:g * PC + o + w], OP.add)
                            nc.scalar.activation(gt, gt, fn,
                                                 bias=cfv(f"bl{l}{g}"))
                        else:
                            nc.scalar.activation(gt, gps[g][:, :w], fn,
                                                 bias=cfv(f"bl{l}{g}"))
                        ga[g] = gt
                    cs = c_tile[:, o:o + w]
                    if t == 0:
                        nc.vector.tensor_tensor(cs, ga[0], ga[2], OP.mult)
                    else:
                        fc_ = t_y[:, o:o + w]
                        nc.vector.tensor_tensor(fc_, ga[1], cs, OP.mult)
                        igg = t_e[:, o:o + w]
                        nc.vector.tensor_tensor(igg, ga[0], ga[2], OP.mult)
                        nc.vector.tensor_tensor(cs, fc_, igg, OP.add)
                    tc_ = gact[2][:, :w]
                    nc.scalar.activation(tc_, cs, AF.Tanh)
                    nc.vector.tensor_tensor(out_bf[:, o:o + w], ga[3], tc_,
                                            OP.mult)

            # ================= pipeline =================

            # ---- input projection: h0 = elu(bn(W_in @ x)) ----
            ps = psBig.tile([128, PC], F32, tag="big", name="ps_proj")
            w_in = wbf("w_in")
            for g in range(XG):
                for j in range(XGK):
                    t = g * XGK + j
                    mm_acc(ps, w_in[:, t * 128:(t + 1) * 128],
                           xq_tiles[g][:, j * PC:(j + 1) * PC],
                           start=(t == 0), stop=(t == FT - 1))
            elu_from(h_bf[0], ps, cfv("sc_in"), cfv("sh_in"))

            # ---- gather h0 (two node-halves) ----
            send_half(0, 0, h_bf[0])
            send_half(0, 1, h_bf[0])

            # ---- embed projection in the collective window ----
            ps_e = psBig.tile([128, PC], F32, tag="big", name="ps_e")
            for ti in range(2):
                mm_acc(ps_e[:64, :], wbf(f"wemb{ti}"),
                       embs[:, ti * PC:(ti + 1) * PC],
                       start=(ti == 0), stop=(ti == 1))
            elu_from(e_bf, ps_e[:64, :], cfv64("sc_emb"), cfv64("sh_emb"))
            pa_stack.close()

            zh_stack = ExitStack()
            zhp = zh_stack.enter_context(tc.tile_pool(name="zhp", bufs=1))

            # ---- GNN layers ----
            for l in range(L):
                ps_agg = psBig.tile([128, PC], F32, tag="big", name="ps_agg")
                hnat0 = recv_half(l, 0)
                agg_half(ps_agg, hnat0, 0, start=True)
                hnat1 = recv_half(l, 1)
                agg_half(ps_agg, hnat1, 1, start=False)
                nc.vector.tensor_tensor(neigh_bf, ps_agg, rsb, OP.mult)

                # GS linear: relu(W_self @ h + W_neigh @ neigh + b)
                ps_gs = psBig.tile([128, PC], F32, tag="big", name="ps_gs")
                mm_acc(ps_gs, wbf(f"wgs_s{l}"), h_bf[l], start=True, stop=False)
                mm_acc(ps_gs, wbf(f"wgs_n{l}"), neigh_bf, start=False, stop=True)
                hrelu = t_r
                nc.scalar.activation(hrelu, ps_gs, AF.Relu,
                                     bias=cfv(f"bgs{l}"), scale=1.0)
                norm_cols(h_bf[l + 1], hrelu)

                if l == 0:
                    # send h1 for the next layer, then fill the collective
                    # window with the t=0 LSTM cells and the hh-precompute
                    # for the critical-path t=1 cell
                    send_half(1, 0, h_bf[1])
                    send_half(1, 1, h_bf[1])
                    lstm_cell(0, 0, h_bf[1], None, c_st[0], o0_bf)
                    lstm_cell(1, 0, o0_bf, None, c_st[1], p0_bf)
                    zhh0 = zhp.tile([128, 4 * PC], BF16, tag="zhh")
                    for (o, w) in CHUNKS:
                        for g in range(4):
                            ps = psG.tile([128, 512], F32, tag="gate",
                                          name="ps_zhh")
                            nc.tensor.matmul(
                                ps[:, :w],
                                wbf("whh0")[:, g * 128:(g + 1) * 128],
                                o0_bf[:, o:o + w], start=True, stop=True)
                            nc.vector.tensor_copy(
                                zhh0[:, g * PC + o:g * PC + o + w],
                                ps[:, :w])

            # ---- remaining LSTM cells ----
            lstm_cell(0, 1, h_bf[2], o0_bf, c_st[0], o1_bf, zhh=zhh0)
            lstm_cell(1, 1, o1_bf, p0_bf, c_st[1], p1_bf)

            # ---- post: JK mean -> bn/elu ; fc ; logits ; log_softmax ----
            hsum = t_r
            nc.vector.tensor_tensor(hsum, p0_bf, p1_bf, OP.add)
            elu_from(hpost_bf, hsum, cfv("sc_in_h"), cfv("sh_in2"))

            ps_fa = psBig.tile([128, PC], F32, tag="big", name="ps_fa")
            mm_acc(ps_fa, wbf("wfc_a")[:, :128], hpost_bf, start=True, stop=False)
            mm_acc(ps_fa, wbf("wfc_b", 64)[:, :128], e_bf, start=False, stop=True)
            elu_from(hfca_bf, ps_fa, cfv("sc_fc_a"), cfv("sh_fc_a"))
            ps_fb = psBig.tile([128, PC], F32, tag="big", name="ps_fb")
            mm_acc(ps_fb[:64, :], wbf("wfc_a")[:, 128:], hpost_bf,
                   start=True, stop=False)
            mm_acc(ps_fb[:64, :], wbf("wfc_b", 64)[:, 128:], e_bf,
                   start=False, stop=True)
            elu_from(hfcb_bf, ps_fb[:64, :], cfv64("sc_fc_b"), cfv64("sh_fc_b"))

            # logits per node-tile; log_softmax without max-subtraction
            bout = cf[:, CF_OFF["bout"]:CF_OFF["bout"] + NOUT]
            for it in range(IT):
                ps_lg = psSml.tile([128, 512], F32, tag="bc",
                                   name="ps_lg")[:, :NOUT]
                nc.tensor.matmul(ps_lg, hfca_bf[:, it * 128:(it + 1) * 128],
                                 wbf("wout_a"), start=True, stop=False)
                nc.tensor.matmul(ps_lg, hfcb_bf[:, it * 128:(it + 1) * 128],
                                 wbf("wout_b", 64), start=False, stop=True)
                nc.vector.tensor_tensor(
                    outall[:, it * NOUT:(it + 1) * NOUT], ps_lg, bout, OP.add)
            for it in range(IT):
                nc.scalar.activation(
                    exs, outall[:, it * NOUT:(it + 1) * NOUT], AF.Exp,
                    accum_out=sem[:, it:it + 1])
            nc.scalar.activation(lse, sem, AF.Ln)
            for it in range(IT):
                sl = outall[:, it * NOUT:(it + 1) * NOUT]
                nc.vector.tensor_scalar(sl, sl, lse[:, it:it + 1], None,
                                        OP.subtract)

            nc.scalar.dma_start(
                d_out.ap().rearrange("(t p) c -> p t c", p=128),
                outall.rearrange("p (t c) -> p t c", t=IT))
            zh_stack.close()

    nc.compile()
    return nc


# --------------------------------------------------------------------------
# host side
# --------------------------------------------------------------------------

def _stage_inputs(
    x, embed, adj, W_in, b_in, bn_in_g, bn_in_b, bn_in_rm, bn_in_rv,
    W_gs, b_gs, Wih0, Whh0, bih0, bhh0, Wih1, Whh1, bih1, bhh1,
    W_emb, b_emb, bn_emb_g, bn_emb_b, bn_emb_rm, bn_emb_rv,
    W_fc, b_fc, bn_fc_g, bn_fc_b, bn_fc_rm, bn_fc_rv, W_out, b_out,
):
    x = np.asarray(x, np.float32)
    embed = np.asarray(embed, np.float32)
    adj = np.asarray(adj, np.float32)

    def bn_fold(g, b, rm, rv, lin_b=None):
        g = np.asarray(g, np.float32); b = np.asarray(b, np.float32)
        rm = np.asarray(rm, np.float32); rv = np.asarray(rv, np.float32)
        sc = g / np.sqrt(rv + BN_EPS)
        base = lin_b if lin_b is not None else 0.0
        shv = sc * (base - rm) + b
        return _f32(sc), _f32(shv)

    sc_in, sh_in = bn_fold(bn_in_g, bn_in_b, bn_in_rm, bn_in_rv,
                           np.asarray(b_in, np.float32))
    _, sh_in2 = bn_fold(bn_in_g, bn_in_b, bn_in_rm, bn_in_rv)
    sc_emb, sh_emb = bn_fold(bn_emb_g, bn_emb_b, bn_emb_rm, bn_emb_rv,
                             np.asarray(b_emb, np.float32))
    sc_fc, sh_fc = bn_fold(bn_fc_g, bn_fc_b, bn_fc_rm, bn_fc_rv,
                           np.asarray(b_fc, np.float32))

    # ---- packed bf16 consts ----
    cbf = np.zeros((128, CBF_W), ml_dtypes.bfloat16)

    def put(nm, arr):
        arr = np.asarray(arr, np.float32)
        p, w = arr.shape
        cbf[:p, CBF_OFF[nm]:CBF_OFF[nm] + w] = _bf(arr)

    W_in = np.asarray(W_in, np.float32)
    w_inT = np.zeros((FPAD, NH), np.float32)
    w_inT[:NFEAT] = W_in.T
    # w_in sbuf layout: [p, t*128 + j] = W_inT[t*128 + p, j]
    put("w_in", w_inT.reshape(FT, 128, NH).transpose(1, 0, 2).reshape(128, FPAD))

    W_gs = np.asarray(W_gs, np.float32)
    for l in range(L):
        put(f"wgs_s{l}", W_gs[l][:, :NH].T)
        put(f"wgs_n{l}", W_gs[l][:, NH:].T)
    put("wih0", np.asarray(Wih0, np.float32).T)
    put("whh0", np.asarray(Whh0, np.float32).T)
    put("wih1", np.asarray(Wih1, np.float32).T)
    put("whh1", np.asarray(Whh1, np.float32).T)
    W_emb = np.asarray(W_emb, np.float32)
    put("wemb0", W_emb[:, :128].T)
    put("wemb1", W_emb[:, 128:].T)
    W_fc = np.asarray(W_fc, np.float32)
    put("wfc_a", W_fc[:, :128].T)      # [128, 192]
    put("wfc_b", W_fc[:, 128:].T)      # [64, 192]
    W_out = np.asarray(W_out, np.float32)
    put("wout_a", W_out[:, :128].T)
    put("wout_b", W_out[:, 128:].T)

    # ---- packed f32 consts ----
    cfp = np.zeros((128, CF_W), np.float32)

    def putf(nm, vec, p=128):
        v = np.asarray(vec, np.float32).reshape(-1)
        cfp[:p, CF_OFF[nm]] = v

    putf("sc_in", sc_in); putf("sh_in", sh_in)
    putf("sc_in_h", 0.5 * sc_in); putf("sh_in2", sh_in2)
    b_gs = np.asarray(b_gs, np.float32)
    putf("bgs0", b_gs[0]); putf("bgs1", b_gs[1])
    bl0 = np.asarray(bih0, np.float32) + np.asarray(bhh0, np.float32)
    bl1 = np.asarray(bih1, np.float32) + np.asarray(bhh1, np.float32)
    for g in range(4):
        putf(f"bl0{g}", bl0[g * NH:(g + 1) * NH])
        putf(f"bl1{g}", bl1[g * NH:(g + 1) * NH])
    putf("sc_emb", sc_emb, 64); putf("sh_emb", sh_emb, 64)
    putf("sc_fc_a", sc_fc[:128]); putf("sh_fc_a", sh_fc[:128])
    putf("sc_fc_b", sc_fc[128:], 64); putf("sh_fc_b", sh_fc[128:], 64)
    cfp[:, CF_OFF["bout"]:CF_OFF["bout"] + NOUT] = np.asarray(
        b_out, np.float32)[None, :]

    shared = {"cbf": cbf, "cf": cfp}

    rowsum = adj.sum(axis=1)                     # fp32, exact rows
    in_maps = []
    for c in range(NC):
        rows = slice(c * NPC, (c + 1) * NPC)
        # transposed fp8 adjacency shard with padded global node ordering
        adjT = np.zeros((NP, PC), ml_dtypes.float8_e4m3fn)
        blk = _f8(adj[rows].T)                   # [10000, 1250]
        for ck in range(NC):
            adjT[ck * PC:ck * PC + NPC, :NPC] = blk[ck * NPC:(ck + 1) * NPC]
        # group-major DMA layout: [g, p, tt*PC + i]
        adjq = np.ascontiguousarray(
            adjT.reshape(AG, AGK, 128, PC).transpose(0, 2, 1, 3)
            .reshape(AG, 128, AGK * PC))

        xT = np.zeros((FPAD, PC), ml_dtypes.float8_e4m3fn)
        xT[:NFEAT, :NPC] = _f8(x[rows].T)
        xq = np.ascontiguousarray(
            xT.reshape(XG, XGK, 128, PC).transpose(0, 2, 1, 3)
            .reshape(XG, 128, XGK * PC))

        embT = np.zeros((2, 128, PC), ml_dtypes.bfloat16)
        embT[:, :, :NPC] = _bf(embed[rows].T.reshape(2, 128, NPC))

        rec = np.zeros((PC,), np.float32)
        rec[:NPC] = 1.0 / rowsum[rows]
        rsb = np.ascontiguousarray(
            np.broadcast_to(_bf(rec)[None, :], (128, PC)))

        m = {"adjq": adjq, "xq": xq, "embT": embT, "rsb": rsb}
        m.update(shared)
        in_maps.append(m)
    return in_maps


def kernel(**inputs) -> np.ndarray:
    global _CACHED_NC, LAST_RESULT
    in_maps = _stage_inputs(**inputs)
    if _CACHED_NC is None:
        _CACHED_NC = _build_program()
    nc = _CACHED_NC
    trace = bool(int(os.environ.get("GSAGE_TRACE", "0")))
    res = run_bass_kernel_spmd(
        nc, in_maps, core_ids=list(range(NC)), trace=trace,
    )
    LAST_RESULT = res
    out = np.concatenate(
        [res.results[c]["out"][:NPC] for c in range(NC)], axis=0)
    return np.ascontiguousarray(out, np.float32)


if __name__ == "__main__":
    import reference
    inputs = reference.setup_inputs()
    out = kernel(**{k: np.asarray(v) for k, v in inputs.items()})
    print("out", out.shape, out.dtype)
